# revision 43
# baseline (speedup 1.0000x reference)
"""DispersionLoss kernel for Trainium2 (8 NeuronCores, Bass/Tile).

Reference computation (N=16384, F=64, K=32, C=128):
    bin_mass[f,k]  = sum_n m[n,f,k] + EPS
    SWY[f,k,c]     = sum_n m[n,f,k] * y[n,c]
    cent[f,k,c]    = SWY / bin_mass
    loss_dispersion= sum_fk ( sum_n m*dist2 ) / bin_mass
                   = sum_fk ( A/bin_mass - c_sq - EPS*c_sq/bin_mass )
        where A[f,k] = sum_n m[n,f,k]*|y_n|^2   (algebraic expansion: the
        cross term sum_n m*cross equals bin_mass*c_sq exactly)
    loss_entropy   = sum_fk p*log(p+EPS), p = bin_mass/N
    loss_repulsion = sum_f sum_k exp(-|cent[f,k]-cent[f,k+1]|^2)
    loss_inter     = sum_f sum_{k<j} exp(-|cent[f,k]-cent[f,j]|^2) / F
                   = sum_f (sum_{kj} exp(-pairwise) - K) / 2 / F   (symmetry)

Sharding: over F (8 features per core) -> every loss term decomposes per-f,
so no cross-core collectives are needed; host sums 8 partial scalars.

Each core (inputs arrive fp16, host-packed into DMA-friendly layouts):
  phase 1: [Y | 1] resident in SBUF; per 128-row subtile two fp16 matmuls
    accumulate psum_swyT[c=128, fk=256] += Y.T @ G and
    psum_am[2, fk] += [1 | y_sq].T @ G  (y_sq precomputed in batches).
  phase 2: transpose to bin-major, per-bin stats vectorized across both
    128-bin halves, mean-centered all-pairs distance stage (exp on whole
    blocks, diagonal-block reduces), raw sums DMA'd out; the host sums the
    8 cores' partials and applies the final linear combines in fp64.
"""

import numpy as np

N = 16384
F = 64
K = 32
C = 128
NCORES = 8
F_PER_CORE = F // NCORES          # 8
FK = F_PER_CORE * K               # 256 bins per core
NT = N // 128                     # 128 row-tiles

LAMBDA_ENTROPY = 0.1
LAMBDA_REPULSION = 0.5
LAMBDA_INTER = 0.3
EPS = 1e-8

PG = 8                            # n-subtiles per packed G super-tile
NB = NT // PG                     # 16 super-tiles
YW = C + 1                        # 129: [Y | 1]
SQB = 8                           # subtiles per square/reduce batch

_NC_CACHE = {}


def _pack_g(gc: np.ndarray) -> np.ndarray:
    """(N, FK) -> (NB*128, PG*FK): row p of block b holds subtile rows
    [b*PG*128 + t*128 + p for t in range(PG)] concatenated."""
    return np.ascontiguousarray(
        gc.reshape(NB, PG, 128, FK).transpose(0, 2, 1, 3).reshape(NB * 128, PG * FK)
    )


def _pack_y(yo: np.ndarray) -> np.ndarray:
    """(N, YW) -> (128, NT*YW): partition p holds rows [s*128+p for s] concat."""
    return np.ascontiguousarray(
        yo.reshape(NT, 128, YW).transpose(1, 0, 2).reshape(128, NT * YW)
    )


def _finalize(parts: np.ndarray):
    """parts: (ncores, 8) raw per-core sums
    [wv0, wv1, ent0, ent1, en_tot, en_inv, e_allsum, 0]."""
    r = parts.astype(np.float64).sum(axis=0)
    disp = r[0] + r[1]
    ent = r[2] + r[3]
    rep = r[4] - r[5]
    inter = (r[6] - F * K) / (2.0 * F)
    tot = disp + LAMBDA_ENTROPY * ent + LAMBDA_REPULSION * rep + LAMBDA_INTER * inter
    return tuple(np.float32(v) for v in (tot, disp, ent, rep, inter))


def _build_nc(mode: str):
    import concourse.bacc as bacc
    import concourse.tile as tile
    from concourse import mybir

    f32 = mybir.dt.float32
    fin = {"f32": mybir.dt.float32, "f32r": mybir.dt.float32r,
           "f16": mybir.dt.float16}[mode]

    nc = bacc.Bacc("TRN2", target_bir_lowering=False, debug=False,
                   enable_asserts=False)
    # host-packed layouts (see _pack_g/_pack_y)
    g_dram = nc.dram_tensor("g", (NB * 128, PG * FK), fin, kind="ExternalInput").ap()
    y_dram = nc.dram_tensor("y", (128, NT * YW), fin, kind="ExternalInput").ap()
    out_dram = nc.dram_tensor("out", (1, 8), f32, kind="ExternalOutput").ap()

    with tile.TileContext(nc) as tc:
        with (
            tc.tile_pool(name="singles", bufs=1) as singles,
            tc.tile_pool(name="gpool", bufs=12) as gpool,
            tc.tile_pool(name="scr", bufs=2) as scr,
            tc.tile_pool(name="ph2", bufs=1) as ph2,
            tc.tile_pool(name="psacc", bufs=1, space="PSUM") as psacc,
            tc.tile_pool(name="pstmp", bufs=2, space="PSUM") as pstmp,
        ):
            # ---- constants ----
            mi2 = singles.tile([128, 128], f32)          # -2 * identity
            nc.gpsimd.memset(mi2, 0.0)
            nc.gpsimd.affine_select(
                out=mi2, in_=mi2,
                compare_op=mybir.AluOpType.not_equal,
                fill=-2.0, base=0, pattern=[[-1, 128]], channel_multiplier=1,
            )
            ones128 = singles.tile([128, 1], f32)
            nc.gpsimd.memset(ones128, 1.0)
            eps128 = singles.tile([128, 1], f32)
            nc.gpsimd.memset(eps128, EPS)

            id128 = singles.tile([128, 128], f32)        # +identity
            nc.gpsimd.memset(id128, 0.0)
            nc.gpsimd.affine_select(
                out=id128, in_=id128,
                compare_op=mybir.AluOpType.not_equal,
                fill=1.0, base=0, pattern=[[-1, 128]], channel_multiplier=1,
            )

            # ---- [Y | 1] resident (128 x NT*YW); chunks DMA'd on the
            # scalar queue, interleaved with the main loop so the cold-start
            # backlog stays small.  Chunk j covers subtiles 16j..16j+15.
            yres = singles.tile([128, NT * YW], fin, name="yres")
            # yqt holds per-subtile extras stationaries [1 | y_sq] at cols
            # (2s, 2s+1).  Even cols = 1.0 (one ACT const-fill), odd cols =
            # y_sq computed in SQB-subtile batches.
            yqt = singles.tile([128, 2 * NT], fin, name="yqt")
            yqt3 = yqt.rearrange("p (t two) -> p t two", two=2)

            def emit_square_batch(s0, nb):
                    sqs = scr.tile([128, SQB * YW], f32, tag="sqs", name="sqs")
                    nc.scalar.activation(
                        out=sqs[:, 0:nb * YW], in_=yres[:, s0 * YW:(s0 + nb) * YW],
                        func=mybir.ActivationFunctionType.Square,
                    )
                    red = scr.tile([128, SQB], f32, tag="red", name="red")
                    nc.vector.reduce_sum(
                        red[:, 0:nb],
                        sqs[:, 0:nb * YW].rearrange(
                            "p (t c) -> p t c", c=YW)[:, :, 0:C],
                        axis=mybir.AxisListType.X,
                    )
                    with nc.allow_low_precision(reason="y_sq feeds f32r mm"):
                        nc.vector.tensor_copy(
                            out=yqt3[:, s0:s0 + nb, 1:2],
                            in_=red[:, 0:nb].rearrange(
                                "p (t one) -> p t one", one=1),
                        )

            CHUNKS = [(0, 4), (4, 16), (16, 32), (32, 48), (48, 64),
                      (64, 80), (80, 96), (96, 112), (112, 128)]

            def emit_ychunk(lo, hi):
                nc.scalar.dma_start(
                    out=yres[:, lo * YW:hi * YW],
                    in_=y_dram[:, lo * YW:hi * YW],
                )

            def emit_squares_rng(lo, hi):
                for s0 in range(lo, hi, SQB):
                    emit_square_batch(s0, min(SQB, hi - s0))

            emit_ychunk(*CHUNKS[0])
            with nc.allow_low_precision(reason="extras feed f32r matmul"):
                nc.scalar.activation(
                    out=yqt3[:, :, 0:1],
                    in_=yres[:, 0:NT].rearrange("p (t one) -> p t one", one=1),
                    func=mybir.ActivationFunctionType.Copy,
                    scale=0.0, bias=1.0,
                )
            emit_squares_rng(*CHUNKS[0])
            for lo, hi in CHUNKS[1:]:
                emit_ychunk(lo, hi)
                emit_squares_rng(lo, hi)
            chunk_at_block = {}

            # ---- phase 1: [Y | 1 | y_sq]^T @ G accumulated over subtiles ----
            # Y is stationary; G streams 256 columns so f32r runs at full
            # rate.  Output layout: (c x fk) + (2 x fk).
            ps_swyT = psacc.tile([128, FK], f32)
            ps_am = psacc.tile([2, FK], f32)
            for b in range(NB):
                if b in chunk_at_block:
                    lo, hi = chunk_at_block[b]
                    emit_ychunk(lo, hi)
                    emit_squares_rng(lo, hi)
                g = gpool.tile([128, PG * FK], fin)
                nc.sync.dma_start(out=g, in_=g_dram[b * 128:(b + 1) * 128, :])
                for t in range(PG):
                    s = b * PG + t
                    rhs = g[:, t * FK:(t + 1) * FK]
                    nc.tensor.matmul(
                        ps_swyT, yres[:, s * YW:s * YW + C], rhs,
                        start=(s == 0), stop=(s == NT - 1),
                    )
                    # extras rows: [1 ; y_sq] -> ps_am rows [mass ; A]
                    nc.tensor.matmul(
                        ps_am, yqt[:, 2 * s:2 * s + 2], rhs,
                        start=(s == 0), stop=(s == NT - 1),
                    )

            # ---- transpose back to (fk x [c | mass | A]) layout ----
            swyT_sb = ph2.tile([128, FK], f32)
            nc.scalar.copy(swyT_sb, ps_swyT)
            am_sb = ph2.tile([2, FK], f32)
            nc.scalar.copy(am_sb, ps_am)
            ps = []
            for h in range(2):
                cs = h * 128
                ps_h = psacc.tile([128, 130], f32, tag=f"ps{h}", name=f"ps{h}")
                nc.tensor.matmul(ps_h[:, 0:C], swyT_sb[:, cs:cs + 128], id128,
                                 start=True, stop=True)
                nc.tensor.matmul(ps_h[:, C:C + 2], am_sb[0:2, cs:cs + 128],
                                 id128[0:2, 0:2], start=True, stop=True)
                ps.append(ps_h)

            # ---- per-bin stats, halves vectorized as columns (128 x 2) ----
            mass2 = ph2.tile([128, 2], f32)
            a2 = ph2.tile([128, 2], f32)
            for h in range(2):
                nc.scalar.activation(
                    out=mass2[:, h:h + 1], in_=ps[h][:, 128:129],
                    func=mybir.ActivationFunctionType.Identity,
                    bias=eps128, scale=1.0,
                )
                nc.vector.tensor_copy(a2[:, h:h + 1], ps[h][:, 129:130])
            inv2 = ph2.tile([128, 2], f32)
            nc.vector.reciprocal(inv2, mass2)
            cent = ph2.tile([128, FK], f32)
            for h in range(2):
                nc.vector.tensor_scalar_mul(
                    cent[:, h * 128:(h + 1) * 128],
                    in0=ps[h][:, 0:C], scalar1=inv2[:, h:h + 1],
                )
            csq_scr = scr.tile([128, FK], f32, tag="csqscr")
            nc.vector.tensor_mul(csq_scr, cent, cent)
            c_sq2 = ph2.tile([128, 2], f32)
            nc.vector.reduce_sum(
                c_sq2, csq_scr.rearrange("p (h c) -> p h c", c=128),
                axis=mybir.AxisListType.X,
            )
            # wv = A*inv - c_sq - EPS*c_sq*inv ; ent = p*ln(p+EPS)
            st = ph2.tile([128, 4], f32)
            t0 = ph2.tile([128, 2], f32)
            nc.vector.tensor_mul(t0, a2, inv2)
            nc.vector.tensor_sub(t0, t0, c_sq2)
            t1 = ph2.tile([128, 2], f32)
            nc.vector.tensor_mul(t1, c_sq2, inv2)
            nc.scalar.mul(t1, t1, EPS)
            nc.vector.tensor_sub(st[:, 0:2], t0, t1)
            pp2 = ph2.tile([128, 2], f32)
            nc.scalar.mul(pp2, mass2, 1.0 / N)
            lg2 = ph2.tile([128, 2], f32)
            nc.scalar.activation(
                out=lg2, in_=pp2,
                func=mybir.ActivationFunctionType.Ln,
                bias=eps128, scale=1.0,
            )
            nc.vector.tensor_mul(st[:, 2:4], pp2, lg2)
            ps_st = pstmp.tile([1, 4], f32, tag="pstmp")
            nc.tensor.matmul(ps_st, ones128, st, start=True, stop=True)

            # ---- centT (c x fk) via PE transpose; top/bot rank-2 rows ----
            centT = ph2.tile([128, FK], f32)
            for h in range(2):
                cs = h * 128
                ps_ct = pstmp.tile([128, 128], f32, tag="pstmp", name=f"pc{h}")
                nc.tensor.matmul(ps_ct, cent[:, cs:cs + 128], id128,
                                 start=True, stop=True)
                nc.scalar.copy(centT[:, cs:cs + 128], ps_ct)
            # Pairwise distances are shift-invariant; subtract the per-c mean
            # so dots/c_sq live at ~1e-3 scale instead of ~32 (kills the
            # catastrophic cancellation in pairwise = csq_k + csq_j - 2 dots).
            mu = ph2.tile([128, 1], f32)
            nc.vector.reduce_sum(mu, centT, axis=mybir.AxisListType.X)
            nc.scalar.mul(mu, mu, 1.0 / FK)
            cc = ph2.tile([128, FK], f32)
            nc.vector.tensor_scalar(
                out=cc, in0=centT, scalar1=mu, scalar2=None,
                op0=mybir.AluOpType.subtract,
            )
            cc2s = scr.tile([128, FK], f32, tag="cc2s")
            nc.vector.tensor_mul(cc2s, cc, cc)
            ps_ccr = pstmp.tile([1, FK], f32, tag="pstmp")
            nc.tensor.matmul(ps_ccr, ones128, cc2s, start=True, stop=True)
            ccr_sb = ph2.tile([1, FK], f32)
            nc.scalar.copy(ccr_sb, ps_ccr)
            cq2 = ph2.tile([128, 2], f32)
            for h in range(2):
                ps_cq = pstmp.tile([128, 1], f32, tag="pstmp", name=f"pq{h}")
                nc.tensor.matmul(ps_cq, ccr_sb[0:1, h * 128:(h + 1) * 128],
                                 ones128[0:1, 0:1], start=True, stop=True)
                nc.scalar.copy(cq2[:, h:h + 1], ps_cq)
            # sct5 columns: [1, ccsq_h0, 1, ccsq_h1, 1]
            sct5 = ph2.tile([128, 5], f32)
            nc.vector.tensor_copy(sct5[:, 0:1], ones128)
            nc.vector.tensor_copy(sct5[:, 1:2], cq2[:, 0:1])
            nc.vector.tensor_copy(sct5[:, 2:3], ones128)
            nc.vector.tensor_copy(sct5[:, 3:4], cq2[:, 1:2])
            nc.vector.tensor_copy(sct5[:, 4:5], ones128)
            top = ph2.tile([2, FK], f32)         # [1 ; c_sq]
            bot = ph2.tile([2, FK], f32)         # [-c_sq/2 ; -1/2]
            for h in range(2):
                cs = h * 128
                ps_t2 = pstmp.tile([2, 128], f32, tag="pstmp", name=f"pt{h}")
                nc.tensor.matmul(ps_t2, sct5[:, 2 * h:2 * h + 2], mi2,
                                 start=True, stop=True)
                nc.scalar.mul(top[0:2, cs:cs + 128], ps_t2, -0.5)
                ps_b2 = pstmp.tile([2, 128], f32, tag="pstmp", name=f"pb{h}")
                nc.tensor.matmul(ps_b2, sct5[:, 2 * h + 1:2 * h + 3], mi2,
                                 start=True, stop=True)
                nc.scalar.mul(bot[0:2, cs:cs + 128], ps_b2, 0.25)

            # ---- repulsion: adjacent-bin distances from centT ----
            dd = ph2.tile([128, FK - 1], f32)
            nc.vector.tensor_sub(dd, centT[:, 0:FK - 1], centT[:, 1:FK])
            nc.vector.tensor_mul(dd, dd, dd)
            ps_nd = pstmp.tile([1, FK - 1], f32, tag="pstmp")
            nc.tensor.matmul(ps_nd, ones128, dd, start=True, stop=True)
            en = ph2.tile([1, FK - 1], f32)
            en_tot = ph2.tile([1, 1], f32)
            nc.scalar.activation(
                out=en, in_=ps_nd, func=mybir.ActivationFunctionType.Exp,
                scale=-1.0, accum_out=en_tot,
            )
            inv_view = en[0:1, 0:(F_PER_CORE - 1) * K].rearrange(
                "p (a b) -> p a b", b=K
            )[:, :, K - 1:K]
            inv_sum = ph2.tile([1, 1], f32)
            nc.vector.reduce_sum(inv_sum, inv_view, axis=mybir.AxisListType.XY)

            # ---- inter: psq[k,j] = dots - (c_sq[k]+c_sq[j])/2 for ALL global
            # pairs; E = exp(2*psq) of the whole block (all entries are valid
            # distances, no overflow), then DVE-reduce only the diagonal
            # (same-f) blocks.
            erows = ph2.tile([128, 2], f32)
            for q in range(2):
                psq = pstmp.tile([128, FK], f32, tag="pwq", name=f"psq{q}")
                nc.tensor.matmul(psq, cc[:, q * 128:(q + 1) * 128], cc,
                                 start=True, stop=False)
                nc.tensor.matmul(psq, top[:, q * 128:(q + 1) * 128], bot,
                                 start=False, stop=True)
                e_full = scr.tile([128, FK], f32, tag="efull", name=f"ef{q}")
                nc.scalar.activation(
                    out=e_full, in_=psq,
                    func=mybir.ActivationFunctionType.Exp, scale=2.0,
                )
                for fl in range(4):
                    fg = q * 4 + fl
                    nc.vector.reduce_sum(
                        erows[32 * fl:32 * fl + 32, q:q + 1],
                        e_full[32 * fl:32 * fl + 32, fg * 32:fg * 32 + 32],
                        axis=mybir.AxisListType.X,
                    )
            ecol = ph2.tile([128, 1], f32)
            nc.vector.reduce_sum(ecol, erows, axis=mybir.AxisListType.X)
            ps_i = pstmp.tile([1, 1], f32, tag="pstmp")
            nc.tensor.matmul(ps_i, ones128, ecol, start=True, stop=True)

            # ---- raw outputs; host finishes the linear combines ----
            # res = [wv0, wv1, ent0, ent1, en_tot, en_inv, e_allsum, 0]
            res = ph2.tile([1, 8], f32)
            nc.gpsimd.memset(res, 0.0)
            nc.scalar.copy(res[0:1, 0:4], ps_st)
            nc.vector.tensor_copy(res[0:1, 4:5], en_tot)
            nc.vector.tensor_copy(res[0:1, 5:6], inv_sum)
            nc.scalar.copy(res[0:1, 6:7], ps_i)
            nc.sync.dma_start(out=out_dram, in_=res)

    nc.compile()
    return nc


def get_nc(mode: str = "f16"):
    if mode not in _NC_CACHE:
        _NC_CACHE[mode] = _build_nc(mode)
    return _NC_CACHE[mode]


def kernel(membership: np.ndarray, teacher_preds: np.ndarray, _trace: bool = False,
           _mode: str = "f16"):
    from concourse.bass_utils import run_bass_kernel_spmd

    np_in = np.float16 if _mode == "f16" else np.float32
    m = np.asarray(membership, dtype=np_in).reshape(N, F * K)
    y = np.asarray(teacher_preds, dtype=np_in)
    y = _pack_y(np.concatenate([y, np.ones((N, 1), dtype=np_in)], axis=1))

    nc = get_nc(_mode)
    in_maps = []
    for i in range(NCORES):
        in_maps.append({
            "g": _pack_g(m[:, i * FK:(i + 1) * FK]),
            "y": y,
        })
    res = run_bass_kernel_spmd(
        nc, in_maps, core_ids=list(range(NCORES)), trace=_trace,
    )
    parts = np.stack(
        [np.asarray(res.results[i]["out"][0], dtype=np.float64) for i in range(NCORES)]
    )
    out = _finalize(parts)
    if _trace:
        return out, res
    return out


if __name__ == "__main__":
    rng = np.random.default_rng(0)
    mem = rng.random((N, F, K), dtype=np.float32)
    tp = rng.random((N, C), dtype=np.float32)
    print(kernel(mem, tp))


# revision 44
# speedup vs baseline: 1.0350x; 1.0350x over previous
"""DispersionLoss kernel for Trainium2 (8 NeuronCores, Bass/Tile).

Reference computation (N=16384, F=64, K=32, C=128):
    bin_mass[f,k]  = sum_n m[n,f,k] + EPS
    SWY[f,k,c]     = sum_n m[n,f,k] * y[n,c]
    cent[f,k,c]    = SWY / bin_mass
    loss_dispersion= sum_fk ( sum_n m*dist2 ) / bin_mass
                   = sum_fk ( A/bin_mass - c_sq - EPS*c_sq/bin_mass )
        where A[f,k] = sum_n m[n,f,k]*|y_n|^2   (algebraic expansion: the
        cross term sum_n m*cross equals bin_mass*c_sq exactly)
    loss_entropy   = sum_fk p*log(p+EPS), p = bin_mass/N
    loss_repulsion = sum_f sum_k exp(-|cent[f,k]-cent[f,k+1]|^2)
    loss_inter     = sum_f sum_{k<j} exp(-|cent[f,k]-cent[f,j]|^2) / F
                   = sum_f (sum_{kj} exp(-pairwise) - K) / 2 / F   (symmetry)

Sharding: over F (8 features per core) -> every loss term decomposes per-f,
so no cross-core collectives are needed; host sums 8 partial scalars.

Each core (inputs arrive fp16, host-packed into DMA-friendly layouts):
  phase 1: [Y | 1] resident in SBUF; per 128-row subtile two fp16 matmuls
    accumulate psum_swyT[c=128, fk=256] += Y.T @ G and
    psum_am[2, fk] += [1 | y_sq].T @ G  (y_sq precomputed in batches).
  phase 2: transpose to bin-major, per-bin stats vectorized across both
    128-bin halves, mean-centered all-pairs distance stage (exp on whole
    blocks, diagonal-block reduces), raw sums DMA'd out; the host sums the
    8 cores' partials and applies the final linear combines in fp64.
"""

import numpy as np

N = 16384
F = 64
K = 32
C = 128
NCORES = 8
F_PER_CORE = F // NCORES          # 8
FK = F_PER_CORE * K               # 256 bins per core
NT = N // 128                     # 128 row-tiles

LAMBDA_ENTROPY = 0.1
LAMBDA_REPULSION = 0.5
LAMBDA_INTER = 0.3
EPS = 1e-8

PG = 8                            # n-subtiles per packed G super-tile
NB = NT // PG                     # 16 super-tiles
YW = C + 1                        # 129: [Y | 1]
SQB = 8                           # subtiles per square/reduce batch

_NC_CACHE = {}


def _pack_g(gc: np.ndarray) -> np.ndarray:
    """(N, FK) -> (NB*128, PG*FK): row p of block b holds subtile rows
    [b*PG*128 + t*128 + p for t in range(PG)] concatenated."""
    return np.ascontiguousarray(
        gc.reshape(NB, PG, 128, FK).transpose(0, 2, 1, 3).reshape(NB * 128, PG * FK)
    )


def _pack_y(yo: np.ndarray) -> np.ndarray:
    """(N, YW) -> (128, NT*YW): partition p holds rows [s*128+p for s] concat."""
    return np.ascontiguousarray(
        yo.reshape(NT, 128, YW).transpose(1, 0, 2).reshape(128, NT * YW)
    )


def _finalize(parts: np.ndarray):
    """parts: (ncores, 8) raw per-core sums
    [wv0, wv1, ent0, ent1, en_tot, en_inv, e_allsum, 0]."""
    r = parts.astype(np.float64).sum(axis=0)
    disp = r[0] + r[1]
    ent = r[2] + r[3]
    rep = r[4] - r[5]
    inter = (r[6] - F * K) / (2.0 * F)
    tot = disp + LAMBDA_ENTROPY * ent + LAMBDA_REPULSION * rep + LAMBDA_INTER * inter
    return tuple(np.float32(v) for v in (tot, disp, ent, rep, inter))


def _build_nc(mode: str):
    import concourse.bacc as bacc
    import concourse.tile as tile
    from concourse import mybir

    f32 = mybir.dt.float32
    fin = {"f32": mybir.dt.float32, "f32r": mybir.dt.float32r,
           "f16": mybir.dt.float16}[mode]

    nc = bacc.Bacc("TRN2", target_bir_lowering=False, debug=False,
                   enable_asserts=False)
    # host-packed layouts (see _pack_g/_pack_y)
    g_dram = nc.dram_tensor("g", (NB * 128, PG * FK), fin, kind="ExternalInput").ap()
    y_dram = nc.dram_tensor("y", (128, NT * YW), fin, kind="ExternalInput").ap()
    out_dram = nc.dram_tensor("out", (1, 8), f32, kind="ExternalOutput").ap()

    with tile.TileContext(nc) as tc:
        with (
            tc.tile_pool(name="singles", bufs=1) as singles,
            tc.tile_pool(name="gpool", bufs=8) as gpool,
            tc.tile_pool(name="scr", bufs=2) as scr,
            tc.tile_pool(name="ph2", bufs=1) as ph2,
            tc.tile_pool(name="psacc", bufs=1, space="PSUM") as psacc,
            tc.tile_pool(name="pstmp", bufs=2, space="PSUM") as pstmp,
        ):
            # ---- constants ----
            mi2 = singles.tile([128, 128], f32)          # -2 * identity
            nc.gpsimd.memset(mi2, 0.0)
            nc.gpsimd.affine_select(
                out=mi2, in_=mi2,
                compare_op=mybir.AluOpType.not_equal,
                fill=-2.0, base=0, pattern=[[-1, 128]], channel_multiplier=1,
            )
            ones128 = singles.tile([128, 1], f32)
            nc.gpsimd.memset(ones128, 1.0)
            eps128 = singles.tile([128, 1], f32)
            nc.gpsimd.memset(eps128, EPS)

            id128 = singles.tile([128, 128], f32)        # +identity
            nc.gpsimd.memset(id128, 0.0)
            nc.gpsimd.affine_select(
                out=id128, in_=id128,
                compare_op=mybir.AluOpType.not_equal,
                fill=1.0, base=0, pattern=[[-1, 128]], channel_multiplier=1,
            )

            # ---- [Y | 1] resident (128 x NT*YW); chunks DMA'd on the
            # scalar queue, interleaved with the main loop so the cold-start
            # backlog stays small.  Chunk j covers subtiles 16j..16j+15.
            yres = singles.tile([128, NT * YW], fin, name="yres")
            # yqt holds per-subtile extras stationaries [1 | y_sq] at cols
            # (2s, 2s+1).  Even cols = 1.0 (one ACT const-fill), odd cols =
            # y_sq computed in SQB-subtile batches.
            yqt = singles.tile([128, 2 * NT], fin, name="yqt")
            yqt3 = yqt.rearrange("p (t two) -> p t two", two=2)

            def emit_square_batch(s0, nb):
                    sqs = scr.tile([128, SQB * YW], f32, tag="sqs", name="sqs")
                    nc.scalar.activation(
                        out=sqs[:, 0:nb * YW], in_=yres[:, s0 * YW:(s0 + nb) * YW],
                        func=mybir.ActivationFunctionType.Square,
                    )
                    red = scr.tile([128, SQB], f32, tag="red", name="red")
                    nc.vector.reduce_sum(
                        red[:, 0:nb],
                        sqs[:, 0:nb * YW].rearrange(
                            "p (t c) -> p t c", c=YW)[:, :, 0:C],
                        axis=mybir.AxisListType.X,
                    )
                    with nc.allow_low_precision(reason="y_sq feeds f32r mm"):
                        nc.vector.tensor_copy(
                            out=yqt3[:, s0:s0 + nb, 1:2],
                            in_=red[:, 0:nb].rearrange(
                                "p (t one) -> p t one", one=1),
                        )

            CHUNKS = [(0, 4), (4, 16), (16, 32), (32, 48), (48, 64),
                      (64, 80), (80, 96), (96, 112), (112, 128)]

            def emit_ychunk(lo, hi):
                nc.scalar.dma_start(
                    out=yres[:, lo * YW:hi * YW],
                    in_=y_dram[:, lo * YW:hi * YW],
                )

            def emit_squares_rng(lo, hi):
                for s0 in range(lo, hi, SQB):
                    emit_square_batch(s0, min(SQB, hi - s0))

            emit_ychunk(*CHUNKS[0])
            with nc.allow_low_precision(reason="extras feed f32r matmul"):
                nc.scalar.activation(
                    out=yqt3[:, :, 0:1],
                    in_=yres[:, 0:NT].rearrange("p (t one) -> p t one", one=1),
                    func=mybir.ActivationFunctionType.Copy,
                    scale=0.0, bias=1.0,
                )
            emit_squares_rng(*CHUNKS[0])
            for lo, hi in CHUNKS[1:]:
                emit_ychunk(lo, hi)
                emit_squares_rng(lo, hi)
            chunk_at_block = {}

            # ---- phase 1: [Y | 1 | y_sq]^T @ G accumulated over subtiles ----
            # Y is stationary; G streams 256 columns so f32r runs at full
            # rate.  Output layout: (c x fk) + (2 x fk).
            ps_swyT = psacc.tile([128, FK], f32)
            ps_am = psacc.tile([2, FK], f32)
            for b in range(NB):
                if b in chunk_at_block:
                    lo, hi = chunk_at_block[b]
                    emit_ychunk(lo, hi)
                    emit_squares_rng(lo, hi)
                g = gpool.tile([128, PG * FK], fin)
                nc.sync.dma_start(out=g, in_=g_dram[b * 128:(b + 1) * 128, :])
                for t in range(PG):
                    s = b * PG + t
                    rhs = g[:, t * FK:(t + 1) * FK]
                    nc.tensor.matmul(
                        ps_swyT, yres[:, s * YW:s * YW + C], rhs,
                        start=(s == 0), stop=(s == NT - 1),
                    )
                    # extras rows: [1 ; y_sq] -> ps_am rows [mass ; A]
                    nc.tensor.matmul(
                        ps_am, yqt[:, 2 * s:2 * s + 2], rhs,
                        start=(s == 0), stop=(s == NT - 1),
                    )

            # ---- transpose back to (fk x [c | mass | A]) layout ----
            swyT_sb = ph2.tile([128, FK], f32)
            nc.scalar.copy(swyT_sb, ps_swyT)
            am_sb = ph2.tile([2, FK], f32)
            nc.scalar.copy(am_sb, ps_am)
            ps = []
            for h in range(2):
                cs = h * 128
                ps_h = psacc.tile([128, 130], f32, tag=f"ps{h}", name=f"ps{h}")
                nc.tensor.matmul(ps_h[:, 0:C], swyT_sb[:, cs:cs + 128], id128,
                                 start=True, stop=True)
                nc.tensor.matmul(ps_h[:, C:C + 2], am_sb[0:2, cs:cs + 128],
                                 id128[0:2, 0:2], start=True, stop=True)
                ps.append(ps_h)

            # ---- per-bin stats, halves vectorized as columns (128 x 2) ----
            mass2 = ph2.tile([128, 2], f32)
            a2 = ph2.tile([128, 2], f32)
            for h in range(2):
                nc.scalar.activation(
                    out=mass2[:, h:h + 1], in_=ps[h][:, 128:129],
                    func=mybir.ActivationFunctionType.Identity,
                    bias=eps128, scale=1.0,
                )
                nc.vector.tensor_copy(a2[:, h:h + 1], ps[h][:, 129:130])
            inv2 = ph2.tile([128, 2], f32)
            nc.vector.reciprocal(inv2, mass2)
            cent = ph2.tile([128, FK], f32)
            for h in range(2):
                nc.vector.tensor_scalar_mul(
                    cent[:, h * 128:(h + 1) * 128],
                    in0=ps[h][:, 0:C], scalar1=inv2[:, h:h + 1],
                )
            csq_scr = scr.tile([128, FK], f32, tag="csqscr")
            nc.vector.tensor_mul(csq_scr, cent, cent)
            c_sq2 = ph2.tile([128, 2], f32)
            nc.vector.reduce_sum(
                c_sq2, csq_scr.rearrange("p (h c) -> p h c", c=128),
                axis=mybir.AxisListType.X,
            )
            # wv = A*inv - c_sq - EPS*c_sq*inv ; ent = p*ln(p+EPS)
            st = ph2.tile([128, 4], f32)
            t0 = ph2.tile([128, 2], f32)
            nc.vector.tensor_mul(t0, a2, inv2)
            nc.vector.tensor_sub(t0, t0, c_sq2)
            t1 = ph2.tile([128, 2], f32)
            nc.vector.tensor_mul(t1, c_sq2, inv2)
            nc.scalar.mul(t1, t1, EPS)
            nc.vector.tensor_sub(st[:, 0:2], t0, t1)
            pp2 = ph2.tile([128, 2], f32)
            nc.scalar.mul(pp2, mass2, 1.0 / N)
            lg2 = ph2.tile([128, 2], f32)
            nc.scalar.activation(
                out=lg2, in_=pp2,
                func=mybir.ActivationFunctionType.Ln,
                bias=eps128, scale=1.0,
            )
            nc.vector.tensor_mul(st[:, 2:4], pp2, lg2)
            ps_st = pstmp.tile([1, 4], f32, tag="pstmp")
            nc.tensor.matmul(ps_st, ones128, st, start=True, stop=True)

            # ---- centT (c x fk) via PE transpose; top/bot rank-2 rows ----
            centT = ph2.tile([128, FK], f32)
            for h in range(2):
                cs = h * 128
                ps_ct = pstmp.tile([128, 128], f32, tag="pstmp", name=f"pc{h}")
                nc.tensor.matmul(ps_ct, cent[:, cs:cs + 128], id128,
                                 start=True, stop=True)
                nc.scalar.copy(centT[:, cs:cs + 128], ps_ct)
            # Pairwise distances are shift-invariant; subtract the per-c mean
            # so dots/c_sq live at ~1e-3 scale instead of ~32 (kills the
            # catastrophic cancellation in pairwise = csq_k + csq_j - 2 dots).
            mu = ph2.tile([128, 1], f32)
            nc.vector.reduce_sum(mu, centT, axis=mybir.AxisListType.X)
            nc.scalar.mul(mu, mu, 1.0 / FK)
            cc = ph2.tile([128, FK], f32)
            nc.vector.tensor_scalar(
                out=cc, in0=centT, scalar1=mu, scalar2=None,
                op0=mybir.AluOpType.subtract,
            )
            cc2s = scr.tile([128, FK], f32, tag="cc2s")
            nc.vector.tensor_mul(cc2s, cc, cc)
            ps_ccr = pstmp.tile([1, FK], f32, tag="pstmp")
            nc.tensor.matmul(ps_ccr, ones128, cc2s, start=True, stop=True)
            ccr_sb = ph2.tile([1, FK], f32)
            nc.scalar.copy(ccr_sb, ps_ccr)
            cq2 = ph2.tile([128, 2], f32)
            for h in range(2):
                ps_cq = pstmp.tile([128, 1], f32, tag="pstmp", name=f"pq{h}")
                nc.tensor.matmul(ps_cq, ccr_sb[0:1, h * 128:(h + 1) * 128],
                                 ones128[0:1, 0:1], start=True, stop=True)
                nc.scalar.copy(cq2[:, h:h + 1], ps_cq)
            # sct5 columns: [1, ccsq_h0, 1, ccsq_h1, 1]
            sct5 = ph2.tile([128, 5], f32)
            nc.vector.tensor_copy(sct5[:, 0:1], ones128)
            nc.vector.tensor_copy(sct5[:, 1:2], cq2[:, 0:1])
            nc.vector.tensor_copy(sct5[:, 2:3], ones128)
            nc.vector.tensor_copy(sct5[:, 3:4], cq2[:, 1:2])
            nc.vector.tensor_copy(sct5[:, 4:5], ones128)
            top = ph2.tile([2, FK], f32)         # [1 ; c_sq]
            bot = ph2.tile([2, FK], f32)         # [-c_sq/2 ; -1/2]
            for h in range(2):
                cs = h * 128
                ps_t2 = pstmp.tile([2, 128], f32, tag="pstmp", name=f"pt{h}")
                nc.tensor.matmul(ps_t2, sct5[:, 2 * h:2 * h + 2], mi2,
                                 start=True, stop=True)
                nc.scalar.mul(top[0:2, cs:cs + 128], ps_t2, -0.5)
                ps_b2 = pstmp.tile([2, 128], f32, tag="pstmp", name=f"pb{h}")
                nc.tensor.matmul(ps_b2, sct5[:, 2 * h + 1:2 * h + 3], mi2,
                                 start=True, stop=True)
                nc.scalar.mul(bot[0:2, cs:cs + 128], ps_b2, 0.25)

            # ---- repulsion: adjacent-bin distances from centT ----
            dd = ph2.tile([128, FK - 1], f32)
            nc.vector.tensor_sub(dd, centT[:, 0:FK - 1], centT[:, 1:FK])
            nc.vector.tensor_mul(dd, dd, dd)
            ps_nd = pstmp.tile([1, FK - 1], f32, tag="pstmp")
            nc.tensor.matmul(ps_nd, ones128, dd, start=True, stop=True)
            en = ph2.tile([1, FK - 1], f32)
            en_tot = ph2.tile([1, 1], f32)
            nc.scalar.activation(
                out=en, in_=ps_nd, func=mybir.ActivationFunctionType.Exp,
                scale=-1.0, accum_out=en_tot,
            )
            inv_view = en[0:1, 0:(F_PER_CORE - 1) * K].rearrange(
                "p (a b) -> p a b", b=K
            )[:, :, K - 1:K]
            inv_sum = ph2.tile([1, 1], f32)
            nc.vector.reduce_sum(inv_sum, inv_view, axis=mybir.AxisListType.XY)

            # ---- inter: psq[k,j] = dots - (c_sq[k]+c_sq[j])/2 for ALL global
            # pairs; E = exp(2*psq) of the whole block (all entries are valid
            # distances, no overflow), then DVE-reduce only the diagonal
            # (same-f) blocks.
            erows = ph2.tile([128, 2], f32)
            for q in range(2):
                psq = pstmp.tile([128, FK], f32, tag="pwq", name=f"psq{q}")
                nc.tensor.matmul(psq, cc[:, q * 128:(q + 1) * 128], cc,
                                 start=True, stop=False)
                nc.tensor.matmul(psq, top[:, q * 128:(q + 1) * 128], bot,
                                 start=False, stop=True)
                e_full = scr.tile([128, FK], f32, tag="efull", name=f"ef{q}")
                nc.scalar.activation(
                    out=e_full, in_=psq,
                    func=mybir.ActivationFunctionType.Exp, scale=2.0,
                )
                for fl in range(4):
                    fg = q * 4 + fl
                    nc.vector.reduce_sum(
                        erows[32 * fl:32 * fl + 32, q:q + 1],
                        e_full[32 * fl:32 * fl + 32, fg * 32:fg * 32 + 32],
                        axis=mybir.AxisListType.X,
                    )
            ecol = ph2.tile([128, 1], f32)
            nc.vector.reduce_sum(ecol, erows, axis=mybir.AxisListType.X)
            ps_i = pstmp.tile([1, 1], f32, tag="pstmp")
            nc.tensor.matmul(ps_i, ones128, ecol, start=True, stop=True)

            # ---- raw outputs; host finishes the linear combines ----
            # res = [wv0, wv1, ent0, ent1, en_tot, en_inv, e_allsum, 0]
            res = ph2.tile([1, 8], f32)
            nc.gpsimd.memset(res, 0.0)
            nc.scalar.copy(res[0:1, 0:4], ps_st)
            nc.vector.tensor_copy(res[0:1, 4:5], en_tot)
            nc.vector.tensor_copy(res[0:1, 5:6], inv_sum)
            nc.scalar.copy(res[0:1, 6:7], ps_i)
            nc.sync.dma_start(out=out_dram, in_=res)

    nc.compile()
    return nc


def get_nc(mode: str = "f16"):
    if mode not in _NC_CACHE:
        _NC_CACHE[mode] = _build_nc(mode)
    return _NC_CACHE[mode]


def kernel(membership: np.ndarray, teacher_preds: np.ndarray, _trace: bool = False,
           _mode: str = "f16"):
    from concourse.bass_utils import run_bass_kernel_spmd

    np_in = np.float16 if _mode == "f16" else np.float32
    m = np.asarray(membership, dtype=np_in).reshape(N, F * K)
    y = np.asarray(teacher_preds, dtype=np_in)
    y = _pack_y(np.concatenate([y, np.ones((N, 1), dtype=np_in)], axis=1))

    nc = get_nc(_mode)
    in_maps = []
    for i in range(NCORES):
        in_maps.append({
            "g": _pack_g(m[:, i * FK:(i + 1) * FK]),
            "y": y,
        })
    res = run_bass_kernel_spmd(
        nc, in_maps, core_ids=list(range(NCORES)), trace=_trace,
    )
    parts = np.stack(
        [np.asarray(res.results[i]["out"][0], dtype=np.float64) for i in range(NCORES)]
    )
    out = _finalize(parts)
    if _trace:
        return out, res
    return out


if __name__ == "__main__":
    rng = np.random.default_rng(0)
    mem = rng.random((N, F, K), dtype=np.float32)
    tp = rng.random((N, C), dtype=np.float32)
    print(kernel(mem, tp))


# revision 45
# speedup vs baseline: 1.0421x; 1.0068x over previous
"""DispersionLoss kernel for Trainium2 (8 NeuronCores, Bass/Tile).

Reference computation (N=16384, F=64, K=32, C=128):
    bin_mass[f,k]  = sum_n m[n,f,k] + EPS
    SWY[f,k,c]     = sum_n m[n,f,k] * y[n,c]
    cent[f,k,c]    = SWY / bin_mass
    loss_dispersion= sum_fk ( sum_n m*dist2 ) / bin_mass
                   = sum_fk ( A/bin_mass - c_sq - EPS*c_sq/bin_mass )
        where A[f,k] = sum_n m[n,f,k]*|y_n|^2   (algebraic expansion: the
        cross term sum_n m*cross equals bin_mass*c_sq exactly)
    loss_entropy   = sum_fk p*log(p+EPS), p = bin_mass/N
    loss_repulsion = sum_f sum_k exp(-|cent[f,k]-cent[f,k+1]|^2)
    loss_inter     = sum_f sum_{k<j} exp(-|cent[f,k]-cent[f,j]|^2) / F
                   = sum_f (sum_{kj} exp(-pairwise) - K) / 2 / F   (symmetry)

Sharding: over F (8 features per core) -> every loss term decomposes per-f,
so no cross-core collectives are needed; host sums 8 partial scalars.

Each core (inputs arrive fp16, host-packed into DMA-friendly layouts):
  phase 1: [Y | 1] resident in SBUF; per 128-row subtile two fp16 matmuls
    accumulate psum_swyT[c=128, fk=256] += Y.T @ G and
    psum_am[2, fk] += [1 | y_sq].T @ G  (y_sq precomputed in batches).
  phase 2: transpose to bin-major, per-bin stats vectorized across both
    128-bin halves, mean-centered all-pairs distance stage (exp on whole
    blocks, diagonal-block reduces), raw sums DMA'd out; the host sums the
    8 cores' partials and applies the final linear combines in fp64.
"""

import numpy as np

N = 16384
F = 64
K = 32
C = 128
NCORES = 8
F_PER_CORE = F // NCORES          # 8
FK = F_PER_CORE * K               # 256 bins per core
NT = N // 128                     # 128 row-tiles

LAMBDA_ENTROPY = 0.1
LAMBDA_REPULSION = 0.5
LAMBDA_INTER = 0.3
EPS = 1e-8

PG = 8                            # n-subtiles per packed G super-tile
NB = NT // PG                     # 16 super-tiles
YW = C + 1                        # 129: [Y | 1]
SQB = 8                           # subtiles per square/reduce batch

_NC_CACHE = {}


def _pack_g(gc: np.ndarray) -> np.ndarray:
    """(N, FK) -> (NB*128, PG*FK): row p of block b holds subtile rows
    [b*PG*128 + t*128 + p for t in range(PG)] concatenated."""
    return np.ascontiguousarray(
        gc.reshape(NB, PG, 128, FK).transpose(0, 2, 1, 3).reshape(NB * 128, PG * FK)
    )


def _pack_y(yo: np.ndarray) -> np.ndarray:
    """(N, YW) -> (128, NT*YW): partition p holds rows [s*128+p for s] concat."""
    return np.ascontiguousarray(
        yo.reshape(NT, 128, YW).transpose(1, 0, 2).reshape(128, NT * YW)
    )


def _finalize(parts: np.ndarray):
    """parts: (ncores, 8) raw per-core sums
    [wv0, wv1, ent0, ent1, en_tot, en_inv, e_allsum, 0]."""
    r = parts.astype(np.float64).sum(axis=0)
    disp = r[0] + r[1]
    ent = r[2] + r[3]
    rep = r[4] - r[5]
    inter = (r[6] - F * K) / (2.0 * F)
    tot = disp + LAMBDA_ENTROPY * ent + LAMBDA_REPULSION * rep + LAMBDA_INTER * inter
    return tuple(np.float32(v) for v in (tot, disp, ent, rep, inter))


def _build_nc(mode: str):
    import concourse.bacc as bacc
    import concourse.tile as tile
    from concourse import mybir

    f32 = mybir.dt.float32
    fin = {"f32": mybir.dt.float32, "f32r": mybir.dt.float32r,
           "f16": mybir.dt.float16}[mode]

    nc = bacc.Bacc("TRN2", target_bir_lowering=False, debug=False,
                   enable_asserts=False)
    # host-packed layouts (see _pack_g/_pack_y)
    g_dram = nc.dram_tensor("g", (NB * 128, PG * FK), fin, kind="ExternalInput").ap()
    y_dram = nc.dram_tensor("y", (128, NT * YW), fin, kind="ExternalInput").ap()
    out_dram = nc.dram_tensor("out", (1, 8), f32, kind="ExternalOutput").ap()

    with tile.TileContext(nc) as tc:
        with (
            tc.tile_pool(name="singles", bufs=1) as singles,
            tc.tile_pool(name="gpool", bufs=8) as gpool,
            tc.tile_pool(name="scr", bufs=2) as scr,
            tc.tile_pool(name="ph2", bufs=1) as ph2,
            tc.tile_pool(name="psacc", bufs=1, space="PSUM") as psacc,
            tc.tile_pool(name="pstmp", bufs=2, space="PSUM") as pstmp,
        ):
            # ---- constants ----
            mi2 = singles.tile([128, 128], f32)          # -2 * identity
            nc.gpsimd.memset(mi2, 0.0)
            nc.gpsimd.affine_select(
                out=mi2, in_=mi2,
                compare_op=mybir.AluOpType.not_equal,
                fill=-2.0, base=0, pattern=[[-1, 128]], channel_multiplier=1,
            )
            ones128 = singles.tile([128, 1], f32)
            nc.gpsimd.memset(ones128, 1.0)
            eps128 = singles.tile([128, 1], f32)
            nc.gpsimd.memset(eps128, EPS)

            id128 = singles.tile([128, 128], f32)        # +identity
            nc.gpsimd.memset(id128, 0.0)
            nc.gpsimd.affine_select(
                out=id128, in_=id128,
                compare_op=mybir.AluOpType.not_equal,
                fill=1.0, base=0, pattern=[[-1, 128]], channel_multiplier=1,
            )

            # ---- [Y | 1] resident (128 x NT*YW); chunks DMA'd on the
            # scalar queue, interleaved with the main loop so the cold-start
            # backlog stays small.  Chunk j covers subtiles 16j..16j+15.
            yres = singles.tile([128, NT * YW], fin, name="yres")
            # yqt holds per-subtile extras stationaries [1 | y_sq] at cols
            # (2s, 2s+1).  Even cols = 1.0 (one ACT const-fill), odd cols =
            # y_sq computed in SQB-subtile batches.
            yqt = singles.tile([128, 2 * NT], fin, name="yqt")
            yqt3 = yqt.rearrange("p (t two) -> p t two", two=2)

            def emit_square_batch(s0, nb):
                    sqs = scr.tile([128, SQB * YW], f32, tag="sqs", name="sqs")
                    nc.scalar.activation(
                        out=sqs[:, 0:nb * YW], in_=yres[:, s0 * YW:(s0 + nb) * YW],
                        func=mybir.ActivationFunctionType.Square,
                    )
                    red = scr.tile([128, SQB], f32, tag="red", name="red")
                    nc.vector.reduce_sum(
                        red[:, 0:nb],
                        sqs[:, 0:nb * YW].rearrange(
                            "p (t c) -> p t c", c=YW)[:, :, 0:C],
                        axis=mybir.AxisListType.X,
                    )
                    with nc.allow_low_precision(reason="y_sq feeds f32r mm"):
                        nc.vector.tensor_copy(
                            out=yqt3[:, s0:s0 + nb, 1:2],
                            in_=red[:, 0:nb].rearrange(
                                "p (t one) -> p t one", one=1),
                        )

            CHUNKS = [(0, 4), (4, 16), (16, 32), (32, 48), (48, 64),
                      (64, 80), (80, 96), (96, 112), (112, 128)]

            def emit_ychunk(lo, hi):
                nc.scalar.dma_start(
                    out=yres[:, lo * YW:hi * YW],
                    in_=y_dram[:, lo * YW:hi * YW],
                )

            def emit_squares_rng(lo, hi):
                for s0 in range(lo, hi, SQB):
                    emit_square_batch(s0, min(SQB, hi - s0))

            emit_ychunk(*CHUNKS[0])
            with nc.allow_low_precision(reason="extras feed f32r matmul"):
                nc.scalar.activation(
                    out=yqt3[:, :, 0:1],
                    in_=yres[:, 0:NT].rearrange("p (t one) -> p t one", one=1),
                    func=mybir.ActivationFunctionType.Copy,
                    scale=0.0, bias=1.0,
                )
            emit_squares_rng(*CHUNKS[0])
            for lo, hi in CHUNKS[1:]:
                emit_ychunk(lo, hi)
                emit_squares_rng(lo, hi)
            chunk_at_block = {}

            # ---- phase 1: [Y | 1 | y_sq]^T @ G accumulated over subtiles ----
            # Y is stationary; G streams 256 columns so f32r runs at full
            # rate.  Output layout: (c x fk) + (2 x fk).
            ps_swyT = psacc.tile([128, FK], f32)
            ps_am = psacc.tile([2, FK], f32)
            for b in range(NB):
                if b in chunk_at_block:
                    lo, hi = chunk_at_block[b]
                    emit_ychunk(lo, hi)
                    emit_squares_rng(lo, hi)
                g = gpool.tile([128, PG * FK], fin)
                nc.sync.dma_start(out=g, in_=g_dram[b * 128:(b + 1) * 128, :])
                for t in range(PG):
                    s = b * PG + t
                    rhs = g[:, t * FK:(t + 1) * FK]
                    nc.tensor.matmul(
                        ps_swyT, yres[:, s * YW:s * YW + C], rhs,
                        start=(s == 0), stop=(s == NT - 1),
                    )
                    # extras rows: [1 ; y_sq] -> ps_am rows [mass ; A]
                    nc.tensor.matmul(
                        ps_am, yqt[:, 2 * s:2 * s + 2], rhs,
                        start=(s == 0), stop=(s == NT - 1),
                    )

            # prefetch the Exp table into its ACT slot (Square's slot) while
            # PE finishes; the Ln table lives in the other slot.
            expwarm = ph2.tile([1, 1], f32)
            nc.scalar.activation(
                out=expwarm, in_=eps128[0:1, 0:1],
                func=mybir.ActivationFunctionType.Exp,
            )

            # ---- transpose back to (fk x [c | mass | A]) layout ----
            swyT_sb = ph2.tile([128, FK], f32)
            nc.scalar.copy(swyT_sb, ps_swyT)
            am_sb = ph2.tile([2, FK], f32)
            nc.scalar.copy(am_sb, ps_am)
            ps = []
            for h in range(2):
                cs = h * 128
                ps_h = psacc.tile([128, 130], f32, tag=f"ps{h}", name=f"ps{h}")
                nc.tensor.matmul(ps_h[:, 0:C], swyT_sb[:, cs:cs + 128], id128,
                                 start=True, stop=True)
                nc.tensor.matmul(ps_h[:, C:C + 2], am_sb[0:2, cs:cs + 128],
                                 id128[0:2, 0:2], start=True, stop=True)
                ps.append(ps_h)

            # ---- per-bin stats, halves vectorized as columns (128 x 2) ----
            mass2 = ph2.tile([128, 2], f32)
            a2 = ph2.tile([128, 2], f32)
            for h in range(2):
                nc.scalar.activation(
                    out=mass2[:, h:h + 1], in_=ps[h][:, 128:129],
                    func=mybir.ActivationFunctionType.Identity,
                    bias=eps128, scale=1.0,
                )
                nc.vector.tensor_copy(a2[:, h:h + 1], ps[h][:, 129:130])
            inv2 = ph2.tile([128, 2], f32)
            nc.vector.reciprocal(inv2, mass2)
            cent = ph2.tile([128, FK], f32)
            for h in range(2):
                nc.vector.tensor_scalar_mul(
                    cent[:, h * 128:(h + 1) * 128],
                    in0=ps[h][:, 0:C], scalar1=inv2[:, h:h + 1],
                )
            csq_scr = scr.tile([128, FK], f32, tag="csqscr")
            nc.vector.tensor_mul(csq_scr, cent, cent)
            c_sq2 = ph2.tile([128, 2], f32)
            nc.vector.reduce_sum(
                c_sq2, csq_scr.rearrange("p (h c) -> p h c", c=128),
                axis=mybir.AxisListType.X,
            )
            # wv = A*inv - c_sq - EPS*c_sq*inv ; ent = p*ln(p+EPS)
            st = ph2.tile([128, 4], f32)
            t0 = ph2.tile([128, 2], f32)
            nc.vector.tensor_mul(t0, a2, inv2)
            nc.vector.tensor_sub(st[:, 0:2], t0, c_sq2)
            pp2 = ph2.tile([128, 2], f32)
            nc.scalar.mul(pp2, mass2, 1.0 / N)
            lg2 = ph2.tile([128, 2], f32)
            nc.scalar.activation(
                out=lg2, in_=pp2,
                func=mybir.ActivationFunctionType.Ln,
                bias=eps128, scale=1.0,
            )
            nc.vector.tensor_mul(st[:, 2:4], pp2, lg2)
            ps_st = pstmp.tile([1, 4], f32, tag="pstmp")
            nc.tensor.matmul(ps_st, ones128, st, start=True, stop=True)

            # ---- cc = centT - cent_bin0 (c x fk), centering fused into the
            # psum->sbuf copy as a per-partition bias.  Distances are
            # shift-invariant; small operands kill the csq+csq-2dots
            # cancellation.
            cc = ph2.tile([128, FK], f32)
            nshift = ph2.tile([128, 1], f32)
            for h in range(2):
                cs = h * 128
                ps_ct = pstmp.tile([128, 128], f32, tag="pstmp", name=f"pc{h}")
                nc.tensor.matmul(ps_ct, cent[:, cs:cs + 128], id128,
                                 start=True, stop=True)
                if h == 0:
                    nc.vector.tensor_scalar_mul(nshift, in0=ps_ct[:, 0:1],
                                                scalar1=-1.0)
                nc.scalar.activation(
                    out=cc[:, cs:cs + 128], in_=ps_ct,
                    func=mybir.ActivationFunctionType.Identity,
                    bias=nshift, scale=1.0,
                )
            cc2s = scr.tile([128, FK], f32, tag="cc2s")
            nc.vector.tensor_mul(cc2s, cc, cc)
            ps_ccr = pstmp.tile([1, FK], f32, tag="pstmp")
            nc.tensor.matmul(ps_ccr, ones128, cc2s, start=True, stop=True)
            ccr_sb = ph2.tile([1, FK], f32)
            nc.scalar.copy(ccr_sb, ps_ccr)
            cq2 = ph2.tile([128, 2], f32)
            for h in range(2):
                ps_cq = pstmp.tile([128, 1], f32, tag="pstmp", name=f"pq{h}")
                nc.tensor.matmul(ps_cq, ccr_sb[0:1, h * 128:(h + 1) * 128],
                                 ones128[0:1, 0:1], start=True, stop=True)
                nc.scalar.copy(cq2[:, h:h + 1], ps_cq)
            # sct5 columns: [1, ccsq_h0, 1, ccsq_h1, 1]
            sct5 = ph2.tile([128, 5], f32)
            nc.vector.tensor_copy(sct5[:, 0:1], ones128)
            nc.vector.tensor_copy(sct5[:, 1:2], cq2[:, 0:1])
            nc.vector.tensor_copy(sct5[:, 2:3], ones128)
            nc.vector.tensor_copy(sct5[:, 3:4], cq2[:, 1:2])
            nc.vector.tensor_copy(sct5[:, 4:5], ones128)
            top = ph2.tile([2, FK], f32)         # [1 ; c_sq]
            bot = ph2.tile([2, FK], f32)         # [-c_sq/2 ; -1/2]
            for h in range(2):
                cs = h * 128
                ps_t2 = pstmp.tile([2, 128], f32, tag="pstmp", name=f"pt{h}")
                nc.tensor.matmul(ps_t2, sct5[:, 2 * h:2 * h + 2], mi2,
                                 start=True, stop=True)
                nc.scalar.mul(top[0:2, cs:cs + 128], ps_t2, -0.5)
                ps_b2 = pstmp.tile([2, 128], f32, tag="pstmp", name=f"pb{h}")
                nc.tensor.matmul(ps_b2, sct5[:, 2 * h + 1:2 * h + 3], mi2,
                                 start=True, stop=True)
                nc.scalar.mul(bot[0:2, cs:cs + 128], ps_b2, 0.25)

            # ---- repulsion: adjacent-bin distances from cc ----
            dd = ph2.tile([128, FK - 1], f32)
            nc.vector.tensor_sub(dd, cc[:, 0:FK - 1], cc[:, 1:FK])
            nc.vector.tensor_mul(dd, dd, dd)
            ps_nd = pstmp.tile([1, FK - 1], f32, tag="pstmp")
            nc.tensor.matmul(ps_nd, ones128, dd, start=True, stop=True)
            en = ph2.tile([1, FK - 1], f32)
            en_tot = ph2.tile([1, 1], f32)
            nc.scalar.activation(
                out=en, in_=ps_nd, func=mybir.ActivationFunctionType.Exp,
                scale=-1.0, accum_out=en_tot,
            )
            inv_view = en[0:1, 0:(F_PER_CORE - 1) * K].rearrange(
                "p (a b) -> p a b", b=K
            )[:, :, K - 1:K]
            inv_sum = ph2.tile([1, 1], f32)
            nc.vector.reduce_sum(inv_sum, inv_view, axis=mybir.AxisListType.XY)

            # ---- inter: psq[k,j] = dots - (c_sq[k]+c_sq[j])/2 for ALL global
            # pairs; E = exp(2*psq) of the whole block (all entries are valid
            # distances, no overflow), then DVE-reduce only the diagonal
            # (same-f) blocks.
            erows = ph2.tile([128, 2], f32)
            for q in range(2):
                psq = pstmp.tile([128, FK], f32, tag="pwq", name=f"psq{q}")
                nc.tensor.matmul(psq, cc[:, q * 128:(q + 1) * 128], cc,
                                 start=True, stop=False)
                nc.tensor.matmul(psq, top[:, q * 128:(q + 1) * 128], bot,
                                 start=False, stop=True)
                e_full = scr.tile([128, FK], f32, tag="efull", name=f"ef{q}")
                nc.scalar.activation(
                    out=e_full, in_=psq,
                    func=mybir.ActivationFunctionType.Exp, scale=2.0,
                )
                for fl in range(4):
                    fg = q * 4 + fl
                    nc.vector.reduce_sum(
                        erows[32 * fl:32 * fl + 32, q:q + 1],
                        e_full[32 * fl:32 * fl + 32, fg * 32:fg * 32 + 32],
                        axis=mybir.AxisListType.X,
                    )
            ecol = ph2.tile([128, 1], f32)
            nc.vector.reduce_sum(ecol, erows, axis=mybir.AxisListType.X)
            ps_i = pstmp.tile([1, 1], f32, tag="pstmp")
            nc.tensor.matmul(ps_i, ones128, ecol, start=True, stop=True)

            # ---- raw outputs; host finishes the linear combines ----
            # res = [wv0, wv1, ent0, ent1, en_tot, en_inv, e_allsum, 0]
            res = ph2.tile([1, 8], f32)
            nc.gpsimd.memset(res, 0.0)
            nc.scalar.copy(res[0:1, 0:4], ps_st)
            nc.vector.tensor_copy(res[0:1, 4:5], en_tot)
            nc.vector.tensor_copy(res[0:1, 5:6], inv_sum)
            nc.scalar.copy(res[0:1, 6:7], ps_i)
            nc.sync.dma_start(out=out_dram, in_=res)

    nc.compile()
    return nc


def get_nc(mode: str = "f16"):
    if mode not in _NC_CACHE:
        _NC_CACHE[mode] = _build_nc(mode)
    return _NC_CACHE[mode]


def kernel(membership: np.ndarray, teacher_preds: np.ndarray, _trace: bool = False,
           _mode: str = "f16"):
    from concourse.bass_utils import run_bass_kernel_spmd

    np_in = np.float16 if _mode == "f16" else np.float32
    m = np.asarray(membership, dtype=np_in).reshape(N, F * K)
    y = np.asarray(teacher_preds, dtype=np_in)
    y = _pack_y(np.concatenate([y, np.ones((N, 1), dtype=np_in)], axis=1))

    nc = get_nc(_mode)
    in_maps = []
    for i in range(NCORES):
        in_maps.append({
            "g": _pack_g(m[:, i * FK:(i + 1) * FK]),
            "y": y,
        })
    res = run_bass_kernel_spmd(
        nc, in_maps, core_ids=list(range(NCORES)), trace=_trace,
    )
    parts = np.stack(
        [np.asarray(res.results[i]["out"][0], dtype=np.float64) for i in range(NCORES)]
    )
    out = _finalize(parts)
    if _trace:
        return out, res
    return out


if __name__ == "__main__":
    rng = np.random.default_rng(0)
    mem = rng.random((N, F, K), dtype=np.float32)
    tp = rng.random((N, C), dtype=np.float32)
    print(kernel(mem, tp))
